# revision 31
# baseline (speedup 1.0000x reference)
"""Local-window MHA (B=4, L=4096, H=1024, 16 heads, window=128) on 8 TRN2 cores.

Sharding: 128 independent windows -> 16 windows/core, data-parallel.
Device kernel (per core, bf16 compute, fp32 PSUM accumulate):
  - qkT[d, t] = WinT.T-family matmul (q rows pre-scaled by 1/sqrt(hd) on host)
  - v[t, d]   natural-layout matmul
  - per window: S^T = k.T-major matmul (lhsT=k, rhs=q; transpose-free),
    exp batched 4 heads/ACT op -> pT (even/odd heads on separate PSUM banks:
    concurrent row-tiled MMs co-writing a bank fault the HW), Z broadcast via
    all-ones matmul, P^T = pT * recip_approx(Z) on DVE, O^T = V.T-major
    matmul (pairs packed via col tiling), out-proj from O^T chunks.
All layout transforms (transposes, bf16 casts) done host-side; biases folded
host-side where linear, per-partition on device for q/k.
"""

import numpy as np
import ml_dtypes

_CACHE = {}

B, L, H = 4, 4096, 1024
NH, HD, P = 16, 64, 128
NWIN = (B * L // P)          # 128 windows total
NCORES = 8
WPC = NWIN // NCORES         # 16 windows per core
NG = 4                       # groups of 4 windows per core
GW = 4                       # windows per group
GT = GW * P                  # 512 tokens per group
HC = H // 128                # 8 h-chunks
DC_QK = 2 * H // 128         # 16 d-chunks for q+k (2048 rows)
BF16 = ml_dtypes.bfloat16


def _build():
    import concourse.bass as bass
    import concourse.mybir as mybir
    import concourse.tile as tile
    from concourse import bacc
    from concourse.alu_op_type import AluOpType

    fp32 = mybir.dt.float32
    bf16 = mybir.dt.bfloat16

    nc = bacc.Bacc("TRN2", target_bir_lowering=False, debug=False)
    xt = nc.dram_tensor("xt", [NG * HC, 128, GT], bf16, kind="ExternalInput")
    winT = nc.dram_tensor("winT", [HC, 128, 3 * H], bf16, kind="ExternalInput")
    woutT = nc.dram_tensor("woutT", [HC, 128, H], bf16, kind="ExternalInput")
    qkb = nc.dram_tensor("qkb", [128, DC_QK], fp32, kind="ExternalInput")
    out = nc.dram_tensor("out", [WPC * P, H], bf16, kind="ExternalOutput")

    with tile.TileContext(nc) as tc:
        with (
            tc.tile_pool(name="wpool", bufs=1) as wpool,
            tc.tile_pool(name="xpool", bufs=16) as xpool,
            tc.tile_pool(name="qkpool", bufs=32) as qkpool,
            tc.tile_pool(name="vpool", bufs=6) as vpool,
            tc.tile_pool(name="ptpool", bufs=8) as ptpool,
            tc.tile_pool(name="ptnpool", bufs=18) as ptnpool,
            tc.tile_pool(name="rbpool", bufs=4) as rbpool,
            tc.tile_pool(name="otppool", bufs=3) as otppool,
            tc.tile_pool(name="outpool", bufs=3) as outpool,
            tc.tile_pool(name="ps512", bufs=2, space="PSUM") as ps512,
            tc.tile_pool(name="psattn", bufs=4, space="PSUM") as psattn,
            tc.tile_pool(name="pspv", bufs=1, space="PSUM") as pspv,
            tc.tile_pool(name="psout", bufs=1, space="PSUM") as psout,
        ):
            # ---- group-0 x first, then weights in need-order ----
            xg0 = []
            for h in range(HC):
                t = xpool.tile([128, GT], bf16, tag="xg")
                nc.sync.dma_start(t[:], xt[h])
                xg0.append(t)
            win_sb = []
            for h in range(HC):
                t = wpool.tile([128, 3 * H], bf16, tag=f"win{h}")
                for s in range(2):
                    cs = slice(s * 1024, (s + 1) * 1024)
                    nc.sync.dma_start(t[:, cs], winT[h][:, cs])
                win_sb.append(t)
            qkb_sb = wpool.tile([128, DC_QK], fp32, tag="qkb")
            nc.sync.dma_start(qkb_sb[:], qkb[:])
            allones = wpool.tile([128, 128], bf16, tag="ones")
            nc.vector.memset(allones[:], 1.0)
            for h in range(HC):
                nc.sync.dma_start(win_sb[h][:, 2 * H:], winT[h][:, 2 * H:])
            wout_sb = []
            for d in range(HC):
                t = wpool.tile([128, H], bf16, tag=f"wout{d}")
                nc.sync.dma_start(t[:], woutT[d])
                wout_sb.append(t)

            for g in range(NG):
                # ---- load x^T for this group ----
                if g == 0:
                    xg = xg0
                else:
                    xg = []
                    for h in range(HC):
                        t = xpool.tile([128, GT], bf16, tag="xg")
                        nc.sync.dma_start(t[:], xt[g * HC + h])
                        xg.append(t)

                # ---- qkT[d, t] : 16 chunks of 128 d-rows ----
                qk_sb = []
                for dc in range(DC_QK):
                    ps = ps512.tile([128, GT], fp32, tag="ps512")
                    for h in range(HC):
                        nc.tensor.matmul(
                            ps[:],
                            win_sb[h][:, dc * 128:(dc + 1) * 128],
                            xg[h][:],
                            start=(h == 0), stop=(h == HC - 1),
                        )
                    sb = qkpool.tile([128, GT], bf16, tag="qk")
                    nc.scalar.activation(
                        sb[:], ps[:], mybir.ActivationFunctionType.Identity,
                        bias=qkb_sb[:, dc:dc + 1],
                    )
                    qk_sb.append(sb)

                # ---- v[t, d] natural layout, per window ----
                v_sb = []
                for w in range(GW):
                    vt = vpool.tile([128, H], bf16, tag="v")
                    for vc in range(2):
                        ps = ps512.tile([128, 512], fp32, tag="ps512")
                        for h in range(HC):
                            nc.tensor.matmul(
                                ps[:],
                                xg[h][:, w * P:(w + 1) * P],
                                win_sb[h][:, 2 * H + vc * 512: 2 * H + (vc + 1) * 512],
                                start=(h == 0), stop=(h == HC - 1),
                            )
                        nc.vector.tensor_copy(vt[:, vc * 512:(vc + 1) * 512], ps[:])
                    v_sb.append(vt)

                # ---- attention front-half (S^T + softmax) per window ----
                ptns = []
                for w in range(GW):
                    ws = slice(w * P, (w + 1) * P)

                    # S^T = exp(k.T q); even/odd heads split across two PSUM
                    # banks so concurrent row-tiled matmuls never co-write a
                    # bank (same-bank concurrent MM groups fault the HW).
                    # Block b holds heads: (b//2)*8 + {0,2,4,6} if b even
                    #                                 {1,3,5,7} if b odd.
                    ptn = []   # normalized P^T, 4 blocks of [128, 512]
                    sts = []
                    for c in range(2):
                        stA = psattn.tile([128, 512], fp32, tag="attn")
                        stB = psattn.tile([128, 512], fp32, tag="attn")
                        for i in range(8):
                            h = c * 8 + i
                            st = stA if i % 2 == 0 else stB
                            col = (i // 2) * P
                            psl = slice((h % 2) * 64, (h % 2) * 64 + 64)
                            kt = qk_sb[8 + h // 2]
                            qt = qk_sb[h // 2]
                            nc.tensor.matmul(
                                st[:, col:col + P],
                                kt[psl, ws], qt[psl, ws],
                                start=True, stop=True,
                            )
                        sts.extend((stA, stB))
                    pts = []
                    for st in sts:
                        pt = ptpool.tile([128, 512], bf16, tag="pt")
                        nc.scalar.activation(
                            pt[:], st[:], mybir.ActivationFunctionType.Exp)
                        pts.append(pt)
                    zbs = []
                    for pt in pts:
                        zb = psattn.tile([128, 512], fp32, tag="attn")
                        nc.tensor.matmul(zb[:], allones[:], pt[:],
                                         start=True, stop=True)
                        zbs.append(zb)
                    rbs = []
                    for zb in zbs:
                        rb = rbpool.tile([128, 512], fp32, tag="rb")
                        nc.vector.reciprocal_approx_fast(rb[:], zb[:])
                        rbs.append(rb)
                    for pt, rb in zip(pts, rbs):
                        pn = ptnpool.tile([128, 512], bf16, tag="ptn")
                        nc.vector.tensor_tensor(pn[:], pt[:], rb[:],
                                                AluOpType.mult)
                        ptn.append(pn)
                    ptns.append(ptn)

                # ---- attention back-half (PV + out-proj) per window ----
                for w in range(GW):
                    gw = g * GW + w
                    ptn = ptns[w]

                    # O^T = V.T-major matmul, head pairs packed per 128
                    # parts; out-proj oc=0's first half accumulates between
                    # the PV halves so it never waits on the second otp copy
                    otp = otppool.tile([128, 1024], bf16, tag="otp")
                    out_sb = outpool.tile([128, H], bf16, tag="osb")
                    ps0 = psout.tile([128, 512], fp32, tag="psout")
                    for half in range(2):
                        pv = pspv.tile([128, 512], fp32, tag="pv")
                        for hh in range(8):
                            h = half * 8 + hh
                            j = hh // 2           # pair within this half
                            sub = hh % 2
                            blk = (h // 8) * 2 + (h % 2)
                            bcol = ((h % 8) // 2) * P
                            nc.tensor.matmul(
                                pv[sub * 64:(sub + 1) * 64,
                                   j * P:(j + 1) * P],
                                v_sb[w][:, h * HD:(h + 1) * HD],
                                ptn[blk][:, bcol:bcol + P],
                                start=True, stop=True,
                            )
                        nc.scalar.copy(otp[:, half * 512:(half + 1) * 512],
                                       pv[:])
                        for j in range(half * 4, half * 4 + 4):
                            nc.tensor.matmul(
                                ps0[:],
                                otp[:, j * 128:(j + 1) * 128],
                                wout_sb[j][:, 0:512],
                                start=(j == 0), stop=(j == 7),
                            )
                    nc.scalar.copy(out_sb[:, 0:512], ps0[:])
                    ps1 = psout.tile([128, 512], fp32, tag="psout")
                    for j in range(8):
                        nc.tensor.matmul(
                            ps1[:],
                            otp[:, j * 128:(j + 1) * 128],
                            wout_sb[j][:, 512:1024],
                            start=(j == 0), stop=(j == 7),
                        )
                    nc.scalar.copy(out_sb[:, 512:1024], ps1[:])
                    for s in range(2):
                        cs = slice(s * 512, (s + 1) * 512)
                        nc.sync.dma_start(out[gw * P:(gw + 1) * P, cs],
                                          out_sb[:, cs])

    nc.compile()
    return nc


def _prep_inputs(x, in_proj_weight, in_proj_bias, out_proj_weight, out_proj_bias,
                 num_heads, window_size):
    assert int(num_heads) == NH and int(window_size) == P
    x = np.asarray(x, dtype=np.float32)
    w_in = np.asarray(in_proj_weight, dtype=np.float32)
    b_in = np.asarray(in_proj_bias, dtype=np.float32)
    w_out = np.asarray(out_proj_weight, dtype=np.float32)
    b_out = np.asarray(out_proj_bias, dtype=np.float32)

    scale = 1.0 / np.sqrt(HD)
    w_in_s = w_in.copy()
    w_in_s[:H] *= scale                      # fold attention scale into q
    winT_np = np.ascontiguousarray(w_in_s.T).astype(BF16).reshape(HC, 128, 3 * H)
    woutT_np = np.ascontiguousarray(w_out.T).astype(BF16).reshape(HC, 128, H)
    qkb_np = np.concatenate([b_in[:H] * scale, b_in[H:2 * H]])
    qkb_np = np.ascontiguousarray(qkb_np.reshape(DC_QK, 128).T).astype(np.float32)
    # v-bias and out-bias are exactly foldable into a constant output shift
    out_shift = (b_in[2 * H:] @ w_out.T + b_out).astype(np.float32)

    xw = x.reshape(NWIN, P, H)
    in_maps = []
    for c in range(NCORES):
        xs = xw[c * WPC:(c + 1) * WPC]                       # [16, 128, 1024]
        xg = xs.reshape(NG, GT, H).transpose(0, 2, 1)        # [4, 1024, 512]
        xt_np = np.ascontiguousarray(xg).astype(BF16).reshape(NG * HC, 128, GT)
        in_maps.append({
            "xt": xt_np, "winT": winT_np, "woutT": woutT_np, "qkb": qkb_np,
        })
    return in_maps, out_shift


def kernel(x, in_proj_weight, in_proj_bias, out_proj_weight, out_proj_bias,
           num_heads, window_size):
    from concourse.bass_utils import run_bass_kernel_spmd

    in_maps, out_shift = _prep_inputs(
        x, in_proj_weight, in_proj_bias, out_proj_weight, out_proj_bias,
        num_heads, window_size)
    if "nc" not in _CACHE:
        _CACHE["nc"] = _build()
    res = run_bass_kernel_spmd(_CACHE["nc"], in_maps, core_ids=list(range(NCORES)))
    outs = np.stack([np.asarray(r["out"]).astype(np.float32)
                     for r in res.results])                  # [8, 2048, 1024]
    full = outs.reshape(B, L, H) + out_shift
    return full.astype(np.float32)


def run_traced(inputs):
    """Profiled run (NTFF -> perfetto); returns BassKernelResults with exec_time_ns."""
    from concourse.bass_utils import run_bass_kernel_spmd

    in_maps, _ = _prep_inputs(**inputs)
    if "nc" not in _CACHE:
        _CACHE["nc"] = _build()
    return run_bass_kernel_spmd(
        _CACHE["nc"], in_maps, core_ids=list(range(NCORES)),
        trace=True, trace_cores=[0])


if __name__ == "__main__":
    rng = np.random.default_rng(0)
    x = rng.standard_normal((B, L, H), dtype=np.float32)
    wi = rng.standard_normal((3 * H, H), dtype=np.float32) * 0.02
    wo = rng.standard_normal((H, H), dtype=np.float32) * 0.02
    o = kernel(x, wi, np.zeros(3 * H, np.float32), wo, np.zeros(H, np.float32), 16, 128)
    print(o.shape, o.dtype)


# revision 33
# speedup vs baseline: 1.2024x; 1.2024x over previous
"""Local-window MHA (B=4, L=4096, H=1024, 16 heads, window=128) on 8 TRN2 cores.

Sharding: 128 independent windows -> 16 windows/core, data-parallel.
Device kernel (per core, bf16 compute, fp32 PSUM accumulate):
  - qkT[d, t] = WinT.T-family matmul (q rows pre-scaled by 1/sqrt(hd) on host)
  - v[t, d]   natural-layout matmul
  - per window: S^T = k.T-major matmul (lhsT=k, rhs=q; transpose-free),
    exp batched 4 heads/ACT op -> pT (even/odd heads on separate PSUM banks:
    concurrent row-tiled MMs co-writing a bank fault the HW), Z broadcast via
    all-ones matmul, P^T = pT * recip_approx(Z) on DVE, O^T = V.T-major
    matmul (pairs packed via col tiling), out-proj from O^T chunks.
All layout transforms (transposes, bf16 casts) done host-side; biases folded
host-side where linear, per-partition on device for q/k.
"""

import numpy as np
import ml_dtypes

_CACHE = {}

B, L, H = 4, 4096, 1024
NH, HD, P = 16, 64, 128
NWIN = (B * L // P)          # 128 windows total
NCORES = 8
WPC = NWIN // NCORES         # 16 windows per core
NG = 4                       # groups of 4 windows per core
GW = 4                       # windows per group
GT = GW * P                  # 512 tokens per group
HC = H // 128                # 8 h-chunks
DC_QK = 2 * H // 128         # 16 d-chunks for q+k (2048 rows)
BF16 = ml_dtypes.bfloat16


def _build():
    import concourse.bass as bass
    import concourse.mybir as mybir
    import concourse.tile as tile
    from concourse import bacc
    from concourse.alu_op_type import AluOpType

    fp32 = mybir.dt.float32
    bf16 = mybir.dt.bfloat16

    nc = bacc.Bacc("TRN2", target_bir_lowering=False, debug=False)
    xt = nc.dram_tensor("xt", [NG * HC, 128, GT], bf16, kind="ExternalInput")
    winT = nc.dram_tensor("winT", [HC, 128, 3 * H], bf16, kind="ExternalInput")
    woutT = nc.dram_tensor("woutT", [HC, 128, H], bf16, kind="ExternalInput")
    qkb = nc.dram_tensor("qkb", [128, DC_QK], fp32, kind="ExternalInput")
    out = nc.dram_tensor("out", [WPC * P, H], bf16, kind="ExternalOutput")

    with tile.TileContext(nc) as tc:
        with (
            tc.tile_pool(name="wpool", bufs=1) as wpool,
            tc.tile_pool(name="xpool", bufs=16) as xpool,
            tc.tile_pool(name="qkpool", bufs=32) as qkpool,
            tc.tile_pool(name="vpool", bufs=6) as vpool,
            tc.tile_pool(name="ptpool", bufs=12) as ptpool,
            tc.tile_pool(name="ptnpool", bufs=8) as ptnpool,
            tc.tile_pool(name="rbpool", bufs=8) as rbpool,
            tc.tile_pool(name="otppool", bufs=3) as otppool,
            tc.tile_pool(name="outpool", bufs=3) as outpool,
            tc.tile_pool(name="ps512", bufs=2, space="PSUM") as ps512,
            tc.tile_pool(name="psattn", bufs=4, space="PSUM") as psattn,
            tc.tile_pool(name="pspv", bufs=1, space="PSUM") as pspv,
            tc.tile_pool(name="psout", bufs=1, space="PSUM") as psout,
        ):
            # ---- group-0 x first, then weights in need-order ----
            xg0 = []
            for h in range(HC):
                t = xpool.tile([128, GT], bf16, tag="xg")
                nc.sync.dma_start(t[:], xt[h])
                xg0.append(t)
            win_sb = []
            for h in range(HC):
                t = wpool.tile([128, 3 * H], bf16, tag=f"win{h}")
                for s in range(2):
                    cs = slice(s * 1024, (s + 1) * 1024)
                    nc.sync.dma_start(t[:, cs], winT[h][:, cs])
                win_sb.append(t)
            qkb_sb = wpool.tile([128, DC_QK], fp32, tag="qkb")
            nc.sync.dma_start(qkb_sb[:], qkb[:])
            allones = wpool.tile([128, 128], bf16, tag="ones")
            nc.vector.memset(allones[:], 1.0)
            for h in range(HC):
                nc.sync.dma_start(win_sb[h][:, 2 * H:], winT[h][:, 2 * H:])
            wout_sb = []
            for d in range(HC):
                t = wpool.tile([128, H], bf16, tag=f"wout{d}")
                nc.sync.dma_start(t[:], woutT[d])
                wout_sb.append(t)

            for g in range(NG):
                # ---- load x^T for this group ----
                if g == 0:
                    xg = xg0
                else:
                    xg = []
                    for h in range(HC):
                        t = xpool.tile([128, GT], bf16, tag="xg")
                        nc.sync.dma_start(t[:], xt[g * HC + h])
                        xg.append(t)

                # ---- qkT[d, t] : 16 chunks of 128 d-rows ----
                qk_sb = []
                for dc in range(DC_QK):
                    ps = ps512.tile([128, GT], fp32, tag="ps512")
                    for h in range(HC):
                        nc.tensor.matmul(
                            ps[:],
                            win_sb[h][:, dc * 128:(dc + 1) * 128],
                            xg[h][:],
                            start=(h == 0), stop=(h == HC - 1),
                        )
                    sb = qkpool.tile([128, GT], bf16, tag="qk")
                    nc.scalar.activation(
                        sb[:], ps[:], mybir.ActivationFunctionType.Identity,
                        bias=qkb_sb[:, dc:dc + 1],
                    )
                    qk_sb.append(sb)

                # ---- v[t, d] natural layout, per window ----
                v_sb = []
                for w in range(GW):
                    vt = vpool.tile([128, H], bf16, tag="v")
                    for vc in range(2):
                        ps = ps512.tile([128, 512], fp32, tag="ps512")
                        for h in range(HC):
                            nc.tensor.matmul(
                                ps[:],
                                xg[h][:, w * P:(w + 1) * P],
                                win_sb[h][:, 2 * H + vc * 512: 2 * H + (vc + 1) * 512],
                                start=(h == 0), stop=(h == HC - 1),
                            )
                        nc.vector.tensor_copy(vt[:, vc * 512:(vc + 1) * 512], ps[:])
                    v_sb.append(vt)

                # ---- attention + out-proj per window ----
                for w in range(GW):
                    gw = g * GW + w
                    ws = slice(w * P, (w + 1) * P)

                    # S^T = exp(k.T q); even/odd heads split across two PSUM
                    # banks so concurrent row-tiled matmuls never co-write a
                    # bank (same-bank concurrent MM groups fault the HW).
                    # Block b holds heads: (b//2)*8 + {0,2,4,6} if b even
                    #                                 {1,3,5,7} if b odd.
                    ptn = []   # normalized P^T, 4 blocks of [128, 512]
                    sts = []
                    for c in range(2):
                        stA = psattn.tile([128, 512], fp32, tag="attn")
                        stB = psattn.tile([128, 512], fp32, tag="attn")
                        for i in range(8):
                            h = c * 8 + i
                            st = stA if i % 2 == 0 else stB
                            col = (i // 2) * P
                            psl = slice((h % 2) * 64, (h % 2) * 64 + 64)
                            kt = qk_sb[8 + h // 2]
                            qt = qk_sb[h // 2]
                            nc.tensor.matmul(
                                st[:, col:col + P],
                                kt[psl, ws], qt[psl, ws],
                                start=True, stop=True,
                            )
                        sts.extend((stA, stB))
                    pts = []
                    for st in sts:
                        pt = ptpool.tile([128, 512], bf16, tag="pt")
                        nc.scalar.activation(
                            pt[:], st[:], mybir.ActivationFunctionType.Exp)
                        pts.append(pt)
                    zbs = []
                    for pt in pts:
                        zb = psattn.tile([128, 512], fp32, tag="attn")
                        nc.tensor.matmul(zb[:], allones[:], pt[:],
                                         start=True, stop=True)
                        zbs.append(zb)
                    rbs = []
                    for zb in zbs:
                        rb = rbpool.tile([128, 512], fp32, tag="rb")
                        nc.vector.reciprocal_approx_fast(rb[:], zb[:])
                        rbs.append(rb)
                    for pt, rb in zip(pts, rbs):
                        pn = ptnpool.tile([128, 512], bf16, tag="ptn")
                        nc.vector.tensor_tensor(pn[:], pt[:], rb[:],
                                                AluOpType.mult)
                        ptn.append(pn)

                    # O^T = V.T-major matmul, head pairs packed per 128
                    # parts; out-proj oc=0's first half accumulates between
                    # the PV halves so it never waits on the second otp copy
                    otp = otppool.tile([128, 1024], bf16, tag="otp")
                    out_sb = outpool.tile([128, H], bf16, tag="osb")
                    ps0 = psout.tile([128, 512], fp32, tag="psout")
                    for half in range(2):
                        pv = pspv.tile([128, 512], fp32, tag="pv")
                        for hh in range(8):
                            h = half * 8 + hh
                            j = hh // 2           # pair within this half
                            sub = hh % 2
                            blk = (h // 8) * 2 + (h % 2)
                            bcol = ((h % 8) // 2) * P
                            nc.tensor.matmul(
                                pv[sub * 64:(sub + 1) * 64,
                                   j * P:(j + 1) * P],
                                v_sb[w][:, h * HD:(h + 1) * HD],
                                ptn[blk][:, bcol:bcol + P],
                                start=True, stop=True,
                            )
                        nc.scalar.copy(otp[:, half * 512:(half + 1) * 512],
                                       pv[:])
                        for j in range(half * 4, half * 4 + 4):
                            nc.tensor.matmul(
                                ps0[:],
                                otp[:, j * 128:(j + 1) * 128],
                                wout_sb[j][:, 0:512],
                                start=(j == 0), stop=(j == 7),
                            )
                    nc.scalar.copy(out_sb[:, 0:512], ps0[:])
                    ps1 = psout.tile([128, 512], fp32, tag="psout")
                    for j in range(8):
                        nc.tensor.matmul(
                            ps1[:],
                            otp[:, j * 128:(j + 1) * 128],
                            wout_sb[j][:, 512:1024],
                            start=(j == 0), stop=(j == 7),
                        )
                    nc.scalar.copy(out_sb[:, 512:1024], ps1[:])
                    for s in range(2):
                        cs = slice(s * 512, (s + 1) * 512)
                        nc.sync.dma_start(out[gw * P:(gw + 1) * P, cs],
                                          out_sb[:, cs])

    nc.compile()
    return nc


def _prep_inputs(x, in_proj_weight, in_proj_bias, out_proj_weight, out_proj_bias,
                 num_heads, window_size):
    assert int(num_heads) == NH and int(window_size) == P
    x = np.asarray(x, dtype=np.float32)
    w_in = np.asarray(in_proj_weight, dtype=np.float32)
    b_in = np.asarray(in_proj_bias, dtype=np.float32)
    w_out = np.asarray(out_proj_weight, dtype=np.float32)
    b_out = np.asarray(out_proj_bias, dtype=np.float32)

    scale = 1.0 / np.sqrt(HD)
    w_in_s = w_in.copy()
    w_in_s[:H] *= scale                      # fold attention scale into q
    winT_np = np.ascontiguousarray(w_in_s.T).astype(BF16).reshape(HC, 128, 3 * H)
    woutT_np = np.ascontiguousarray(w_out.T).astype(BF16).reshape(HC, 128, H)
    qkb_np = np.concatenate([b_in[:H] * scale, b_in[H:2 * H]])
    qkb_np = np.ascontiguousarray(qkb_np.reshape(DC_QK, 128).T).astype(np.float32)
    # v-bias and out-bias are exactly foldable into a constant output shift
    out_shift = (b_in[2 * H:] @ w_out.T + b_out).astype(np.float32)

    xw = x.reshape(NWIN, P, H)
    in_maps = []
    for c in range(NCORES):
        xs = xw[c * WPC:(c + 1) * WPC]                       # [16, 128, 1024]
        xg = xs.reshape(NG, GT, H).transpose(0, 2, 1)        # [4, 1024, 512]
        xt_np = np.ascontiguousarray(xg).astype(BF16).reshape(NG * HC, 128, GT)
        in_maps.append({
            "xt": xt_np, "winT": winT_np, "woutT": woutT_np, "qkb": qkb_np,
        })
    return in_maps, out_shift


def kernel(x, in_proj_weight, in_proj_bias, out_proj_weight, out_proj_bias,
           num_heads, window_size):
    from concourse.bass_utils import run_bass_kernel_spmd

    in_maps, out_shift = _prep_inputs(
        x, in_proj_weight, in_proj_bias, out_proj_weight, out_proj_bias,
        num_heads, window_size)
    if "nc" not in _CACHE:
        _CACHE["nc"] = _build()
    res = run_bass_kernel_spmd(_CACHE["nc"], in_maps, core_ids=list(range(NCORES)))
    outs = np.stack([np.asarray(r["out"]).astype(np.float32)
                     for r in res.results])                  # [8, 2048, 1024]
    full = outs.reshape(B, L, H) + out_shift
    return full.astype(np.float32)


def run_traced(inputs):
    """Profiled run (NTFF -> perfetto); returns BassKernelResults with exec_time_ns."""
    from concourse.bass_utils import run_bass_kernel_spmd

    in_maps, _ = _prep_inputs(**inputs)
    if "nc" not in _CACHE:
        _CACHE["nc"] = _build()
    return run_bass_kernel_spmd(
        _CACHE["nc"], in_maps, core_ids=list(range(NCORES)),
        trace=True, trace_cores=[0])


if __name__ == "__main__":
    rng = np.random.default_rng(0)
    x = rng.standard_normal((B, L, H), dtype=np.float32)
    wi = rng.standard_normal((3 * H, H), dtype=np.float32) * 0.02
    wo = rng.standard_normal((H, H), dtype=np.float32) * 0.02
    o = kernel(x, wi, np.zeros(3 * H, np.float32), wo, np.zeros(H, np.float32), 16, 128)
    print(o.shape, o.dtype)
